# revision 1
# baseline (speedup 1.0000x reference)
"""Trainium2 Bass kernel for nn_CrossAttentionFusion (dense_transformer).

Strategy: pure data parallel over 8 NeuronCores (batch 32768 -> 4096/core).
Token-major layout on chip: batch rows on SBUF partitions, the 4 tokens x 256
features in the free dimension.  Dense matmuls run activation-stationary in
bf16 (fp32 PSUM accumulation, full PE rate + fast weight load); attention
(seq=4, 8 heads x 32 dims) runs on the Vector engine with broadcast access
patterns; LayerNorm uses bn_stats + ScalarE per-partition affine.  LN gains
are folded into the following weight matrices on the host; biases in this
problem are all zero but a general path applies them when nonzero.
"""

import contextlib
import ctypes
import math
import os
import sys
import types
from contextlib import ExitStack

import numpy as np

import concourse.bass as bass
import concourse.tile as tile
from concourse import mybir
from concourse.bass_utils import run_bass_kernel_spmd
from concourse.masks import make_identity


def _install_ntff_hook_shim():
    """Provide antenv.axon_hooks if the image lacks it, so trace=True works.

    Mirrors trn_agent_boot._ntff_profile_via_ctypes: drives NTFF capture via
    the axon PJRT .so's C ABI.  No-op if the real module exists.
    """
    try:
        import antenv.axon_hooks  # noqa: F401
        return
    except ImportError:
        pass
    so_path = "/opt/axon/libaxon_pjrt.so"
    hook = None
    if os.path.exists(so_path):
        try:
            lib = ctypes.CDLL(so_path)
            if hasattr(lib, "axon_start_nrt_profile"):
                lib.axon_start_nrt_profile.argtypes = [
                    ctypes.POINTER(ctypes.c_int64), ctypes.c_size_t]
                lib.axon_start_nrt_profile.restype = ctypes.c_int64
                lib.axon_stop_nrt_profile.argtypes = [ctypes.c_char_p]
                lib.axon_stop_nrt_profile.restype = ctypes.c_int64

                @contextlib.contextmanager
                def _hook(output_dir, device_ids):
                    import jax
                    jax.devices()
                    if device_ids:
                        ids = (ctypes.c_int64 * len(device_ids))(*device_ids)
                        rc = lib.axon_start_nrt_profile(ids, len(device_ids))
                    else:
                        rc = lib.axon_start_nrt_profile(None, 0)
                    if rc != 0:
                        raise RuntimeError(f"axon_start_nrt_profile rc={rc}")
                    try:
                        yield
                    finally:
                        n = lib.axon_stop_nrt_profile(str(output_dir).encode())
                        print(f"ntff profile: {n} file(s) -> {output_dir}",
                              file=sys.stderr)

                hook = _hook
        except OSError:
            pass

    mod = types.ModuleType("antenv.axon_hooks")
    mod.get_axon_ntff_profile_hook = lambda: hook
    mod.set_axon_ntff_profile_hook = lambda h: None
    sys.modules["antenv.axon_hooks"] = mod


_install_ntff_hook_shim()

# Problem shapes (hardcoded per contract).
D, H, HD, FF, L, SYM, B = 256, 8, 32, 256, 3, 64, 32768
NCORES = 8
BC = B // NCORES          # 4096 rows per core
P = 128                   # SBUF partitions
NT = BC // P              # 32 tiles per core
F32 = mybir.dt.float32
F32R = mybir.dt.float32r
BF16 = mybir.dt.bfloat16
AF = mybir.ActivationFunctionType
OP = mybir.AluOpType
EPS = 1e-5
SCALE = 1.0 / math.sqrt(HD)


def _r(ap):
    """View an fp32 AP as float32r for full-rate PE matmuls."""
    return ap.bitcast(F32R)


def _ln_stats(nc, pools, x_ap, ngroups, gsize, psrc=False):
    """Return (rstd [P,ngroups], neg_mu_rstd [P,ngroups]) for LN over gsize.

    x_ap: [P, ngroups, gsize] (or [P, gsize] if ngroups==1).
    """
    work = pools["work"]
    stats = work.tile([P, ngroups, 6], F32, tag="ln_stats")
    if ngroups == 1:
        nc.vector.bn_stats(out=stats[:, 0, :], in_=x_ap)
    else:
        # walrus requires bn_stats output to be exactly 6 elems/partition
        for g in range(ngroups):
            nc.vector.bn_stats(out=stats[:, g, :], in_=x_ap[:, g, :])
    mv = work.tile([P, ngroups, 2], F32, tag="ln_mv")
    for g in range(ngroups):
        nc.vector.bn_aggr(out=mv[:, g, :], in_=stats[:, g, :])
    # rstd = 1/sqrt(var + eps)
    rstd = work.tile([P, ngroups], F32, tag="ln_rstd")
    nc.scalar.activation(
        out=rstd, in_=mv[:, :, 1], func=AF.Sqrt, bias=pools["eps"][:, :1], scale=1.0
    )
    nc.vector.reciprocal(out=rstd, in_=rstd)
    # neg_mu_rstd = -(mu * rstd)
    nmr = work.tile([P, ngroups], F32, tag="ln_nmr")
    nc.vector.scalar_tensor_tensor(
        out=nmr, in0=mv[:, :, 0], scalar=-1.0, in1=rstd, op0=OP.mult, op1=OP.mult
    )
    return rstd, nmr


def _transpose_to_lhst(nc, pools, src_ap, nchunks, tag):
    """PE-transpose src_ap [P, nchunks*128] (fp32) -> SBUF lhsT [128, nchunks, 128].

    Returns the SBUF tile holding x^T chunks: lhsT[:, c, :] = src[:, c*128:(c+1)*128].T
    """
    tp = pools["tpsum"]
    lhst = pools["lhst"].tile([P, nchunks, P], BF16, tag=tag)
    for c0 in range(0, nchunks, 4):
        cn = min(4, nchunks - c0)
        pt = tp.tile([P, 4, P], BF16, tag="tpsum")
        for c in range(cn):
            nc.tensor.transpose(
                pt[:, c, :], src_ap[:, (c0 + c) * P:(c0 + c + 1) * P], pools["identb"]
            )
        nc.scalar.copy(out=lhst[:, c0:c0 + cn, :], in_=pt[:, :cn, :])
    return lhst


def build_kernel(nc, nonzero_bias):
    """Trace the full forward pass for one core (BC rows)."""
    # Per-core data inputs.
    ge = nc.dram_tensor("ge", [BC, D], F32, kind="ExternalInput").ap()
    pe = nc.dram_tensor("pe", [BC, D], F32, kind="ExternalInput").ap()
    pp = nc.dram_tensor("pp", [BC, D], F32, kind="ExternalInput").ap()
    sf = nc.dram_tensor("sf", [BC, SYM], F32, kind="ExternalInput").ap()
    # Folded weights (replicated).
    symw = nc.dram_tensor("symw", [P, D], F32, kind="ExternalInput").ap()  # padded 64->128
    wqkv = nc.dram_tensor("wqkv", [L, 2, P, 3 * D], F32, kind="ExternalInput").ap()
    wo = nc.dram_tensor("wo", [L, 2, P, D], F32, kind="ExternalInput").ap()
    w1 = nc.dram_tensor("w1", [L, 2, P, FF], F32, kind="ExternalInput").ap()
    w2 = nc.dram_tensor("w2", [L, 2, P, D], F32, kind="ExternalInput").ap()
    # Vectors: packed [n, D] table: sym_g, sym_b(+tte2), tte0, tte1, tte3,
    # final_g, final_b, out_g, out_b
    vecs = nc.dram_tensor("vecs", [9, D], F32, kind="ExternalInput").ap()
    bqkv = bmisc = None
    if nonzero_bias:
        bqkv = nc.dram_tensor("bqkv", [L, 3 * D], F32, kind="ExternalInput").ap()
        bmisc = nc.dram_tensor("bmisc", [L, 3, D], F32, kind="ExternalInput").ap()
    out = nc.dram_tensor("out", [BC, D], F32, kind="ExternalOutput").ap()

    with ExitStack() as ctx:
        tc = ctx.enter_context(tile.TileContext(nc))
        singles = ctx.enter_context(tc.tile_pool(name="singles", bufs=1))
        work = ctx.enter_context(tc.tile_pool(name="work", bufs=3))
        xpool = ctx.enter_context(tc.tile_pool(name="xpool", bufs=2))
        qkvpool = ctx.enter_context(tc.tile_pool(name="qkvpool", bufs=1))
        lhstp = ctx.enter_context(tc.tile_pool(name="lhst", bufs=2))
        tpsum = ctx.enter_context(tc.tile_pool(name="tpsum", bufs=2, space="PSUM"))
        mmpsum = ctx.enter_context(tc.tile_pool(name="mmpsum", bufs=2, space="PSUM"))
        opool = ctx.enter_context(tc.tile_pool(name="opool", bufs=2))
        attw = ctx.enter_context(tc.tile_pool(name="attw", bufs=2))

        # ---- load constants ----
        ident = singles.tile([P, P], F32)
        make_identity(nc, ident)
        identb = singles.tile([P, P], BF16)
        make_identity(nc, identb)
        eps_t = singles.tile([P, 1], F32)
        nc.vector.memset(eps_t, EPS)
        symw_sb = singles.tile([P, D], BF16)
        nc.gpsimd.dma_start(out=symw_sb, in_=symw)
        wqkv_sb = singles.tile([P, L, 2, 3 * D], BF16)
        nc.gpsimd.dma_start(out=wqkv_sb, in_=wqkv.transpose([2, 0, 1, 3]))
        wo_sb = singles.tile([P, L, 2, D], BF16)
        nc.gpsimd.dma_start(out=wo_sb, in_=wo.transpose([2, 0, 1, 3]))
        w1_sb = singles.tile([P, L, 2, FF], BF16)
        nc.gpsimd.dma_start(out=w1_sb, in_=w1.transpose([2, 0, 1, 3]))
        w2_sb = singles.tile([P, L, 2, D], BF16)
        nc.gpsimd.dma_start(out=w2_sb, in_=w2.transpose([2, 0, 1, 3]))
        vecs_sb = singles.tile([P, 9, D], F32)
        nc.sync.dma_start(out=vecs_sb, in_=vecs.partition_broadcast(P))
        bqkv_sb = bmisc_sb = None
        if nonzero_bias:
            bqkv_sb = singles.tile([P, L, 3 * D], F32)
            nc.sync.dma_start(out=bqkv_sb, in_=bqkv.partition_broadcast(P))
            bmisc_sb = singles.tile([P, L, 3, D], F32)
            nc.sync.dma_start(out=bmisc_sb, in_=bmisc.partition_broadcast(P))

        pools = {
            "work": work, "tpsum": tpsum, "lhst": lhstp,
            "ident": ident, "identb": identb, "eps": eps_t,
        }
        SYMG, SYMBT, TTE0, TTE1, TTE3 = 0, 1, 2, 3, 4
        FING, FINB, OUTG, OUTB = 5, 6, 7, 8

        for it in range(NT):
            row = it * P
            # ---- build x [P, 4, D] ----
            x = xpool.tile([P, 4, D], F32, tag="x")
            ine = work.tile([P, 3, D], F32, tag="ine")
            nc.sync.dma_start(out=ine[:, 0, :], in_=ge[row:row + P, :])
            nc.sync.dma_start(out=ine[:, 1, :], in_=pe[row:row + P, :])
            nc.sync.dma_start(out=ine[:, 2, :], in_=pp[row:row + P, :])
            sft = work.tile([P, SYM], F32, tag="sft")
            nc.sync.dma_start(out=sft, in_=sf[row:row + P, :])

            nc.vector.tensor_add(x[:, 0, :], ine[:, 0, :], vecs_sb[:, TTE0, :])
            nc.vector.tensor_add(x[:, 1, :], ine[:, 1, :], vecs_sb[:, TTE1, :])
            nc.vector.tensor_add(x[:, 3, :], ine[:, 2, :], vecs_sb[:, TTE3, :])

            # sym branch: LN(sf @ symW) * g + (b + tte2)
            sftp = work.tile([P, P], BF16, tag="sftp")
            nc.vector.memset(sftp[:, SYM:], 0.0)
            nc.vector.tensor_copy(out=sftp[:, :SYM], in_=sft)
            spsum_t = tpsum.tile([P, 4, P], BF16, tag="tpsum", name="spsum")
            spsum = spsum_t[:, 0, :]
            nc.tensor.transpose(spsum, sftp, identb)
            slhst = work.tile([P, P], BF16, tag="slhst")
            nc.scalar.copy(out=slhst, in_=spsum)
            zsym_t = mmpsum.tile([P, 512], F32, tag="mm_d", name="zsym")
            zsym = zsym_t[:, 0:D]
            nc.tensor.matmul(zsym, slhst, symw_sb, start=True, stop=True)
            rstd, nmr = _ln_stats(nc, pools, zsym, 1, D)
            zn = work.tile([P, D], F32, tag="zn")
            nc.scalar.activation(out=zn, in_=zsym, func=AF.Identity,
                                 bias=nmr[:, :1], scale=rstd[:, :1])
            # x2 = zn * symg + (symb + tte2)
            nc.vector.scalar_tensor_tensor(
                out=x[:, 2, :], in0=zn, scalar=1.0, in1=vecs_sb[:, SYMG, :],
                op0=OP.bypass, op1=OP.mult)
            nc.vector.tensor_add(x[:, 2, :], x[:, 2, :], vecs_sb[:, SYMBT, :])

            # ---- transformer layers ----
            for l in range(L):
                # LN1 (gains folded into wqkv)
                rstd, nmr = _ln_stats(nc, pools, x, 4, D)
                t = work.tile([P, 4, D], BF16, tag="t_ln")
                for g in range(4):
                    nc.scalar.activation(out=t[:, g, :], in_=x[:, g, :],
                                         func=AF.Identity,
                                         bias=nmr[:, g:g + 1], scale=rstd[:, g:g + 1])
                # qkv = t @ wqkv  (activation-stationary)
                qkv = qkvpool.tile([P, 4, 3 * D], F32, tag="qkv")
                lhst = _transpose_to_lhst(nc, pools, t.rearrange("p i d -> p (i d)"),
                                          8, "lhst")
                for i in range(4):
                    mp = mmpsum.tile([P, 2, 512], F32, tag="mm_qkv")
                    for c in range(2):
                        nc.tensor.matmul(mp[:, 0, :], lhst[:, 2 * i + c, :],
                                         wqkv_sb[:, l, c, 0:512],
                                         start=(c == 0), stop=(c == 1))
                    for c in range(2):
                        nc.tensor.matmul(mp[:, 1, 0:D], lhst[:, 2 * i + c, :],
                                         wqkv_sb[:, l, c, 512:768],
                                         start=(c == 0), stop=(c == 1))
                    nc.scalar.copy(out=qkv[:, i, 0:512], in_=mp[:, 0, :])
                    nc.scalar.copy(out=qkv[:, i, 512:768], in_=mp[:, 1, 0:D])
                if nonzero_bias:
                    for i in range(4):
                        nc.vector.tensor_add(qkv[:, i, :], qkv[:, i, :],
                                             bqkv_sb[:, l, :])

                # ---- attention ----
                q = qkv[:, :, 0:D].rearrange("p i (h d) -> p i h d", h=H)
                k = qkv[:, :, D:2 * D].rearrange("p i (h d) -> p i h d", h=H)
                v = qkv[:, :, 2 * D:3 * D].rearrange("p i (h d) -> p i h d", h=H)
                # scores[p, i, h, j] = sum_d q[p,i,h,d] * k[p,j,h,d]
                prod = attw.tile([P, 4, 4, H, HD], F32, tag="att_prod")
                qb = q[:, :, None, :, :].to_broadcast((P, 4, 4, H, HD))
                kb = k[:, None, :, :, :].to_broadcast((P, 4, 4, H, HD))
                nc.vector.tensor_tensor(prod, qb, kb, OP.mult)
                sc = work.tile([P, 4, H, 4], F32, tag="att_sc")
                # reduce over d; input iterated (i, j, h); out strided to [i, h, j]
                nc.vector.tensor_reduce(
                    out=sc.transpose([0, 1, 3, 2]), in_=prod,
                    axis=mybir.AxisListType.X, op=OP.add)
                esc = work.tile([P, 4, H, 4], F32, tag="att_esc")
                nc.scalar.activation(out=esc, in_=sc, func=AF.Exp, scale=SCALE)
                den = work.tile([P, 4, H], F32, tag="att_den")
                nc.vector.tensor_reduce(out=den, in_=esc, axis=mybir.AxisListType.X,
                                        op=OP.add)
                nc.vector.reciprocal(out=den, in_=den)
                prob = work.tile([P, 4, H, 4], F32, tag="att_prob")
                nc.vector.tensor_tensor(
                    prob, esc, den[:, :, :, None].to_broadcast((P, 4, H, 4)), OP.mult)
                # o[p, i, h, d] = sum_j prob[p,i,h,j] * v[p,j,h,d]
                pv = attw.tile([P, 4, H, HD, 4], F32, tag="att_pv")
                pb = prob[:, :, :, None, :].to_broadcast((P, 4, H, HD, 4))
                vb = v.transpose([0, 2, 3, 1])[:, None].to_broadcast((P, 4, H, HD, 4))
                nc.vector.tensor_tensor(pv, pb, vb, OP.mult)
                o = opool.tile([P, 4, D], BF16, tag="att_o")
                with nc.allow_low_precision(reason="attn out feeds bf16 matmul"):
                    nc.vector.tensor_reduce(
                        out=o.rearrange("p i (h d) -> p i h d", h=H), in_=pv,
                        axis=mybir.AxisListType.X, op=OP.add)

                # ---- o @ Wo + residual ----
                lhsto = _transpose_to_lhst(nc, pools, o.rearrange("p i d -> p (i d)"),
                                           8, "lhst")
                for i in range(4):
                    mp = mmpsum.tile([P, 512], F32, tag="mm_d")
                    for c in range(2):
                        nc.tensor.matmul(mp[:, 0:D], lhsto[:, 2 * i + c, :],
                                         wo_sb[:, l, c, :],
                                         start=(c == 0), stop=(c == 1))
                    if nonzero_bias:
                        nc.vector.tensor_add(mp[:, 0:D], mp[:, 0:D],
                                             bmisc_sb[:, l, 0, :])
                    nc.vector.tensor_add(x[:, i, :], x[:, i, :], mp[:, 0:D])

                # ---- FF block ----
                rstd, nmr = _ln_stats(nc, pools, x, 4, D)
                t2 = work.tile([P, 4, D], BF16, tag="t2_ln")
                for g in range(4):
                    nc.scalar.activation(out=t2[:, g, :], in_=x[:, g, :],
                                         func=AF.Identity,
                                         bias=nmr[:, g:g + 1], scale=rstd[:, g:g + 1])
                lhst2 = _transpose_to_lhst(nc, pools, t2.rearrange("p i d -> p (i d)"),
                                           8, "lhst")
                gl = work.tile([P, 4, FF], BF16, tag="gelu")
                for i in range(4):
                    mp = mmpsum.tile([P, 512], F32, tag="mm_d")
                    for c in range(2):
                        nc.tensor.matmul(mp[:, 0:FF], lhst2[:, 2 * i + c, :],
                                         w1_sb[:, l, c, :],
                                         start=(c == 0), stop=(c == 1))
                    if nonzero_bias:
                        nc.vector.tensor_add(mp[:, 0:FF], mp[:, 0:FF],
                                             bmisc_sb[:, l, 1, :])
                    nc.scalar.activation(out=gl[:, i, :], in_=mp[:, 0:FF], func=AF.Gelu)
                lhstg = _transpose_to_lhst(nc, pools, gl.rearrange("p i d -> p (i d)"),
                                           8, "lhst")
                for i in range(4):
                    mp = mmpsum.tile([P, 512], F32, tag="mm_d")
                    for c in range(2):
                        nc.tensor.matmul(mp[:, 0:D], lhstg[:, 2 * i + c, :],
                                         w2_sb[:, l, c, :],
                                         start=(c == 0), stop=(c == 1))
                    if nonzero_bias:
                        nc.vector.tensor_add(mp[:, 0:D], mp[:, 0:D],
                                             bmisc_sb[:, l, 2, :])
                    nc.vector.tensor_add(x[:, i, :], x[:, i, :], mp[:, 0:D])

            # ---- tail: final_ln, mean over tokens, out_ln ----
            rstd, nmr = _ln_stats(nc, pools, x, 4, D)
            xt = work.tile([P, 4, D], F32, tag="tail_xt")
            for g in range(4):
                nc.scalar.activation(out=xt[:, g, :], in_=x[:, g, :],
                                     func=AF.Identity,
                                     bias=nmr[:, g:g + 1], scale=rstd[:, g:g + 1])
            s01 = work.tile([P, 2, D], F32, tag="tail_s2")
            nc.vector.tensor_add(s01[:, 0, :], xt[:, 0, :], xt[:, 1, :])
            nc.vector.tensor_add(s01[:, 1, :], xt[:, 2, :], xt[:, 3, :])
            u = work.tile([P, D], F32, tag="tail_u")
            nc.vector.tensor_add(u, s01[:, 0, :], s01[:, 1, :])
            # u = 0.25*u*final_g + final_b
            nc.vector.scalar_tensor_tensor(
                out=u, in0=u, scalar=0.25, in1=vecs_sb[:, FING, :],
                op0=OP.mult, op1=OP.mult)
            nc.vector.tensor_add(u, u, vecs_sb[:, FINB, :])
            rstd, nmr = _ln_stats(nc, pools, u, 1, D)
            un = work.tile([P, D], F32, tag="tail_un")
            nc.scalar.activation(out=un, in_=u, func=AF.Identity,
                                 bias=nmr[:, :1], scale=rstd[:, :1])
            res = opool.tile([P, D], F32, tag="res")
            nc.vector.scalar_tensor_tensor(
                out=res, in0=un, scalar=1.0, in1=vecs_sb[:, OUTG, :],
                op0=OP.bypass, op1=OP.mult)
            nc.vector.tensor_add(res, res, vecs_sb[:, OUTB, :])
            nc.sync.dma_start(out=out[row:row + P, :], in_=res)

    return nc


def _fold_host(inputs):
    """Fold LN gains/biases into weights on the host. Returns weight arrays."""
    f = lambda k: np.asarray(inputs[k], dtype=np.float32)
    wqkv, bqkv = f("Wqkv"), f("bqkv")
    wo, bo = f("Wo"), f("bo")
    w1, b1 = f("W1"), f("b1")
    w2, b2 = f("W2"), f("b2")
    g1, b1n = f("ln1_g"), f("ln1_b")
    g2, b2n = f("ln2_g"), f("ln2_b")

    wqkv_f = np.empty_like(wqkv)
    bqkv_f = np.empty_like(bqkv)
    w1_f = np.empty_like(w1)
    b1_f = np.empty_like(b1)
    for l in range(L):
        wqkv_f[l] = g1[l][:, None] * wqkv[l]
        bqkv_f[l] = b1n[l] @ wqkv[l] + bqkv[l]
        w1_f[l] = g2[l][:, None] * w1[l]
        b1_f[l] = b2n[l] @ w1[l] + b1[l]

    symw = np.zeros((P, D), dtype=np.float32)
    symw[:SYM] = f("sym_W")
    symb = f("sym_b")

    vecs = np.zeros((9, D), dtype=np.float32)
    tte = f("token_type_emb")
    vecs[0] = f("sym_ln_g")
    vecs[1] = f("sym_ln_b") + tte[2]
    vecs[2] = tte[0]
    vecs[3] = tte[1]
    vecs[4] = tte[3]
    vecs[5] = f("final_ln_g")
    vecs[6] = f("final_ln_b")
    vecs[7] = f("out_ln_g")
    vecs[8] = f("out_ln_b")

    bmisc = np.stack([bo, b1_f, b2], axis=1)  # [L, 3, D]
    # sym_b folds into the sym matmul? sym_b is added before LN; approximate by
    # appending to symw via... keep it simple: sym matmul bias handled via vecs?
    # sym_b is a pre-LN bias: z = sf@symW + sym_b; LN removes constant shifts in
    # mean but not exactly (it does: LN(z + c) has same (z-mu) when c is constant
    # across features? No: c varies per feature). Handle nonzero sym_b via bqkv
    # path: we add it with a broadcast add if nonzero.
    nz = any(np.any(a) for a in (bqkv_f, bmisc, symb))
    return dict(symw=symw, symb=symb, wqkv=wqkv_f, bqkv=bqkv_f, wo=wo, w1=w1_f,
                w2=w2, vecs=vecs, bmisc=bmisc, nonzero_bias=bool(nz))


_CACHE = {}


def _get_built(nonzero_bias):
    key = ("k1", nonzero_bias)
    if key not in _CACHE:
        from concourse import bacc
        nc = bacc.Bacc("TRN2", target_bir_lowering=False, debug=False,
                       num_devices=NCORES)
        build_kernel(nc, nonzero_bias)
        nc.compile()
        _CACHE[key] = nc
    return _CACHE[key]


def _chunk_w(w):
    """[L, 256, M] -> [L, 2, 128, M]"""
    Lx, K, M = w.shape
    return np.ascontiguousarray(w.reshape(Lx, 2, P, M))


def kernel(**inputs):
    fold = _fold_host(inputs)
    nzb = fold["nonzero_bias"]
    if np.any(fold["symb"]):
        # rare general path: push sym_b into the padded symw via constant row:
        # append bias as an extra input feature is not available; instead add
        # sym_b to every row after matmul by augmenting... handled by adding
        # sym_b into the matmul via an extra all-ones input column (row 64).
        fold = dict(fold)
        symw = fold["symw"].copy()
        symw[SYM] = fold["symb"]
        fold["symw"] = symw
        _SYM_ONE = True
    else:
        _SYM_ONE = False

    nc = _get_built(nzb)

    ge = np.asarray(inputs["global_emb"], dtype=np.float32)
    pe = np.asarray(inputs["pert_emb"], dtype=np.float32)
    pp = np.asarray(inputs["ppi_feat"], dtype=np.float32)
    sf = np.asarray(inputs["sym_feat"], dtype=np.float32)
    if _SYM_ONE:
        # cannot append ones column without kernel change; fall back: error
        raise NotImplementedError("nonzero sym_b not supported in this build")

    wq = _chunk_w(fold["wqkv"])
    wo = _chunk_w(fold["wo"])
    w1 = _chunk_w(fold["w1"])
    w2 = _chunk_w(fold["w2"])

    in_maps = []
    for c in range(NCORES):
        sl = slice(c * BC, (c + 1) * BC)
        m = {
            "ge": np.ascontiguousarray(ge[sl]),
            "pe": np.ascontiguousarray(pe[sl]),
            "pp": np.ascontiguousarray(pp[sl]),
            "sf": np.ascontiguousarray(sf[sl]),
            "symw": fold["symw"],
            "wqkv": wq, "wo": wo, "w1": w1, "w2": w2,
            "vecs": fold["vecs"],
        }
        if nzb:
            m["bqkv"] = fold["bqkv"].astype(np.float32)
            m["bmisc"] = np.ascontiguousarray(fold["bmisc"].astype(np.float32))
        in_maps.append(m)

    res = run_bass_kernel_spmd(nc, in_maps, core_ids=list(range(NCORES)))
    global LAST_RESULT
    LAST_RESULT = res
    outs = [res.results[c]["out"] for c in range(NCORES)]
    return np.concatenate(outs, axis=0)


LAST_RESULT = None


if __name__ == "__main__":
    rng = np.random.default_rng(0)
    print("smoke build only")
    _get_built(False)
    print("built ok")



# revision 6
# speedup vs baseline: 1.1014x; 1.1014x over previous
"""Trainium2 Bass kernel for nn_CrossAttentionFusion (dense_transformer).

Strategy: pure data parallel over 8 NeuronCores (batch 32768 -> 4096/core).
Token-major layout on chip: batch rows on SBUF partitions, the 4 tokens x 256
features in the free dimension.  Dense matmuls run activation-stationary in
bf16 (fp32 PSUM accumulation, full PE rate + fast weight load); attention
(seq=4, 8 heads x 32 dims) runs on the Vector engine with broadcast access
patterns; LayerNorm uses bn_stats + ScalarE per-partition affine.  LN gains
are folded into the following weight matrices on the host; biases in this
problem are all zero but a general path applies them when nonzero.
"""

import contextlib
import ctypes
import math
import os
import sys
import types
from contextlib import ExitStack

import numpy as np

import concourse.bass as bass
import concourse.tile as tile
from concourse import mybir
from concourse.bass_utils import run_bass_kernel_spmd
from concourse.masks import make_identity


def _install_ntff_hook_shim():
    """Provide antenv.axon_hooks if the image lacks it, so trace=True works.

    Mirrors trn_agent_boot._ntff_profile_via_ctypes: drives NTFF capture via
    the axon PJRT .so's C ABI.  No-op if the real module exists.
    """
    try:
        import antenv.axon_hooks  # noqa: F401
        return
    except ImportError:
        pass
    so_path = "/opt/axon/libaxon_pjrt.so"
    hook = None
    if os.path.exists(so_path):
        try:
            lib = ctypes.CDLL(so_path)
            if hasattr(lib, "axon_start_nrt_profile"):
                lib.axon_start_nrt_profile.argtypes = [
                    ctypes.POINTER(ctypes.c_int64), ctypes.c_size_t]
                lib.axon_start_nrt_profile.restype = ctypes.c_int64
                lib.axon_stop_nrt_profile.argtypes = [ctypes.c_char_p]
                lib.axon_stop_nrt_profile.restype = ctypes.c_int64

                @contextlib.contextmanager
                def _hook(output_dir, device_ids):
                    import jax
                    jax.devices()
                    if device_ids:
                        ids = (ctypes.c_int64 * len(device_ids))(*device_ids)
                        rc = lib.axon_start_nrt_profile(ids, len(device_ids))
                    else:
                        rc = lib.axon_start_nrt_profile(None, 0)
                    if rc != 0:
                        raise RuntimeError(f"axon_start_nrt_profile rc={rc}")
                    try:
                        yield
                    finally:
                        n = lib.axon_stop_nrt_profile(str(output_dir).encode())
                        print(f"ntff profile: {n} file(s) -> {output_dir}",
                              file=sys.stderr)

                hook = _hook
        except OSError:
            pass

    mod = types.ModuleType("antenv.axon_hooks")
    mod.get_axon_ntff_profile_hook = lambda: hook
    mod.set_axon_ntff_profile_hook = lambda h: None
    sys.modules["antenv.axon_hooks"] = mod


_install_ntff_hook_shim()

# Problem shapes (hardcoded per contract).
D, H, HD, FF, L, SYM, B = 256, 8, 32, 256, 3, 64, 32768
NCORES = 8
BC = B // NCORES          # 4096 rows per core
P = 128                   # SBUF partitions
NT = BC // P              # 32 tiles per core
F32 = mybir.dt.float32
F32R = mybir.dt.float32r
BF16 = mybir.dt.bfloat16
AF = mybir.ActivationFunctionType
OP = mybir.AluOpType
EPS = 1e-5
SCALE = 1.0 / math.sqrt(HD)


def _r(ap):
    """View an fp32 AP as float32r for full-rate PE matmuls."""
    return ap.bitcast(F32R)


def _ln_stats(nc, pools, x_ap, ngroups, gsize, psrc=False):
    """Return (rstd [P,ngroups], neg_mu_rstd [P,ngroups]) for LN over gsize.

    x_ap: [P, ngroups, gsize] (or [P, gsize] if ngroups==1).
    """
    work = pools["work"]
    stats = work.tile([P, ngroups, 6], F32, tag="ln_stats")
    if ngroups == 1:
        nc.vector.bn_stats(out=stats[:, 0, :], in_=x_ap)
    else:
        # walrus requires bn_stats output to be exactly 6 elems/partition
        for g in range(ngroups):
            nc.vector.bn_stats(out=stats[:, g, :], in_=x_ap[:, g, :])
    mv = work.tile([P, ngroups, 2], F32, tag="ln_mv")
    for g in range(ngroups):
        nc.vector.bn_aggr(out=mv[:, g, :], in_=stats[:, g, :])
    # rstd = exp(-0.5 * ln(var + eps)); Ln+Exp live in one ACT table set
    # (natural_log_exp_and_others) together with softmax Exp, so the scalar
    # engine never reloads tables between LN and attention.
    lnv = work.tile([P, ngroups], F32, tag="ln_lnv")
    nc.scalar.activation(
        out=lnv, in_=mv[:, :, 1], func=AF.Ln, bias=pools["eps"][:, :1], scale=1.0
    )
    rstd = work.tile([P, ngroups], F32, tag="ln_rstd")
    nc.scalar.activation(out=rstd, in_=lnv, func=AF.Exp, scale=-0.5)
    # neg_mu_rstd = -(mu * rstd)
    nmr = work.tile([P, ngroups], F32, tag="ln_nmr")
    nc.vector.scalar_tensor_tensor(
        out=nmr, in0=mv[:, :, 0], scalar=-1.0, in1=rstd, op0=OP.mult, op1=OP.mult
    )
    return rstd, nmr


def _transpose_to_lhst(nc, pools, src_ap, nchunks, tag):
    """PE-transpose src_ap [P, nchunks*128] (fp32) -> SBUF lhsT [128, nchunks, 128].

    Returns the SBUF tile holding x^T chunks: lhsT[:, c, :] = src[:, c*128:(c+1)*128].T
    """
    tp = pools["tpsum"]
    lhst = pools["lhst"].tile([P, nchunks, P], BF16, tag=tag)
    for c0 in range(0, nchunks, 4):
        cn = min(4, nchunks - c0)
        pt = tp.tile([P, 4, P], BF16, tag="tpsum")
        for c in range(cn):
            nc.tensor.transpose(
                pt[:, c, :], src_ap[:, (c0 + c) * P:(c0 + c + 1) * P], pools["identb"]
            )
        nc.scalar.copy(out=lhst[:, c0:c0 + cn, :], in_=pt[:, :cn, :])
    return lhst


def build_kernel(nc, nonzero_bias):
    """Trace the full forward pass for one core (BC rows)."""
    # Per-core data inputs.
    ge = nc.dram_tensor("ge", [BC, D], F32, kind="ExternalInput").ap()
    pe = nc.dram_tensor("pe", [BC, D], F32, kind="ExternalInput").ap()
    pp = nc.dram_tensor("pp", [BC, D], F32, kind="ExternalInput").ap()
    sf = nc.dram_tensor("sf", [BC, SYM], F32, kind="ExternalInput").ap()
    # Folded weights (replicated).
    symw = nc.dram_tensor("symw", [P, D], F32, kind="ExternalInput").ap()  # padded 64->128
    wqkv = nc.dram_tensor("wqkv", [L, 2, P, 3 * D], F32, kind="ExternalInput").ap()
    wo = nc.dram_tensor("wo", [L, 2, P, D], F32, kind="ExternalInput").ap()
    w1 = nc.dram_tensor("w1", [L, 2, P, FF], F32, kind="ExternalInput").ap()
    w2 = nc.dram_tensor("w2", [L, 2, P, D], F32, kind="ExternalInput").ap()
    # Vectors: packed [n, D] table: sym_g, sym_b(+tte2), tte0, tte1, tte3,
    # final_g, final_b, out_g, out_b
    vecs = nc.dram_tensor("vecs", [9, D], F32, kind="ExternalInput").ap()
    bqkv = bmisc = None
    if nonzero_bias:
        bqkv = nc.dram_tensor("bqkv", [L, 3 * D], F32, kind="ExternalInput").ap()
        bmisc = nc.dram_tensor("bmisc", [L, 3, D], F32, kind="ExternalInput").ap()
    out = nc.dram_tensor("out", [BC, D], F32, kind="ExternalOutput").ap()

    with ExitStack() as ctx:
        tc = ctx.enter_context(tile.TileContext(nc))
        singles = ctx.enter_context(tc.tile_pool(name="singles", bufs=1))
        work = ctx.enter_context(tc.tile_pool(name="work", bufs=3))
        xpool = ctx.enter_context(tc.tile_pool(name="xpool", bufs=2))
        qkvpool = ctx.enter_context(tc.tile_pool(name="qkvpool", bufs=1))
        lhstp = ctx.enter_context(tc.tile_pool(name="lhst", bufs=2))
        tpsum = ctx.enter_context(tc.tile_pool(name="tpsum", bufs=2, space="PSUM"))
        mmpsum = ctx.enter_context(tc.tile_pool(name="mmpsum", bufs=2, space="PSUM"))
        opool = ctx.enter_context(tc.tile_pool(name="opool", bufs=2))
        attw = ctx.enter_context(tc.tile_pool(name="attw", bufs=2))

        # ---- load constants ----
        ident = singles.tile([P, P], F32)
        make_identity(nc, ident)
        identb = singles.tile([P, P], BF16)
        make_identity(nc, identb)
        eps_t = singles.tile([P, 1], F32)
        nc.vector.memset(eps_t, EPS)
        symw_sb = singles.tile([P, D], BF16)
        nc.gpsimd.dma_start(out=symw_sb, in_=symw)
        wqkv_sb = singles.tile([P, L, 2, 3 * D], BF16)
        nc.gpsimd.dma_start(out=wqkv_sb, in_=wqkv.transpose([2, 0, 1, 3]))
        wo_sb = singles.tile([P, L, 2, D], BF16)
        nc.gpsimd.dma_start(out=wo_sb, in_=wo.transpose([2, 0, 1, 3]))
        w1_sb = singles.tile([P, L, 2, FF], BF16)
        nc.gpsimd.dma_start(out=w1_sb, in_=w1.transpose([2, 0, 1, 3]))
        w2_sb = singles.tile([P, L, 2, D], BF16)
        nc.gpsimd.dma_start(out=w2_sb, in_=w2.transpose([2, 0, 1, 3]))
        vecs_sb = singles.tile([P, 9, D], F32)
        nc.sync.dma_start(out=vecs_sb, in_=vecs.partition_broadcast(P))
        bqkv_sb = bmisc_sb = None
        if nonzero_bias:
            bqkv_sb = singles.tile([P, L, 3 * D], F32)
            nc.sync.dma_start(out=bqkv_sb, in_=bqkv.partition_broadcast(P))
            bmisc_sb = singles.tile([P, L, 3, D], F32)
            nc.sync.dma_start(out=bmisc_sb, in_=bmisc.partition_broadcast(P))

        pools = {
            "work": work, "tpsum": tpsum, "lhst": lhstp,
            "ident": ident, "identb": identb, "eps": eps_t,
        }
        SYMG, SYMBT, TTE0, TTE1, TTE3 = 0, 1, 2, 3, 4
        FING, FINB, OUTG, OUTB = 5, 6, 7, 8

        for it in range(NT):
            row = it * P
            # ---- build x [P, 4, D] ----
            x = xpool.tile([P, 4, D], F32, tag="x")
            ine = work.tile([P, 3, D], F32, tag="ine")
            nc.sync.dma_start(out=ine[:, 0, :], in_=ge[row:row + P, :])
            nc.sync.dma_start(out=ine[:, 1, :], in_=pe[row:row + P, :])
            nc.sync.dma_start(out=ine[:, 2, :], in_=pp[row:row + P, :])
            sft = work.tile([P, SYM], F32, tag="sft")
            nc.sync.dma_start(out=sft, in_=sf[row:row + P, :])

            nc.vector.tensor_add(x[:, 0, :], ine[:, 0, :], vecs_sb[:, TTE0, :])
            nc.vector.tensor_add(x[:, 1, :], ine[:, 1, :], vecs_sb[:, TTE1, :])
            nc.vector.tensor_add(x[:, 3, :], ine[:, 2, :], vecs_sb[:, TTE3, :])

            # sym branch: LN(sf @ symW) * g + (b + tte2)
            sftp = work.tile([P, P], BF16, tag="sftp")
            nc.vector.memset(sftp[:, SYM:], 0.0)
            nc.vector.tensor_copy(out=sftp[:, :SYM], in_=sft)
            spsum_t = tpsum.tile([P, 4, P], BF16, tag="tpsum", name="spsum")
            spsum = spsum_t[:, 0, :]
            nc.tensor.transpose(spsum, sftp, identb)
            slhst = work.tile([P, P], BF16, tag="slhst")
            nc.scalar.copy(out=slhst, in_=spsum)
            zsym_t = mmpsum.tile([P, 512], F32, tag="mm_d", name="zsym")
            zsym = zsym_t[:, 0:D]
            nc.tensor.matmul(zsym, slhst, symw_sb, start=True, stop=True)
            rstd, nmr = _ln_stats(nc, pools, zsym, 1, D)
            zn = work.tile([P, D], F32, tag="zn")
            nc.scalar.activation(out=zn, in_=zsym, func=AF.Identity,
                                 bias=nmr[:, :1], scale=rstd[:, :1])
            # x2 = zn * symg + (symb + tte2)
            nc.vector.scalar_tensor_tensor(
                out=x[:, 2, :], in0=zn, scalar=1.0, in1=vecs_sb[:, SYMG, :],
                op0=OP.bypass, op1=OP.mult)
            nc.vector.tensor_add(x[:, 2, :], x[:, 2, :], vecs_sb[:, SYMBT, :])

            # ---- transformer layers ----
            for l in range(L):
                # LN1 (gains folded into wqkv)
                rstd, nmr = _ln_stats(nc, pools, x, 4, D)
                t = work.tile([P, 4, D], BF16, tag="t_ln")
                for g in range(4):
                    nc.scalar.activation(out=t[:, g, :], in_=x[:, g, :],
                                         func=AF.Identity,
                                         bias=nmr[:, g:g + 1], scale=rstd[:, g:g + 1])
                # qkv = t @ wqkv  (activation-stationary).  Each lhsT chunk is
                # loaded once and reused for both output slices (LDW reuse).
                qk = qkvpool.tile([P, 4, 512], BF16, tag="qk")
                vt = qkvpool.tile([P, H, HD, 4], BF16, tag="vt")
                lhst = _transpose_to_lhst(nc, pools, t.rearrange("p i d -> p (i d)"),
                                          8, "lhst")
                for i in range(4):
                    mp = mmpsum.tile([P, 2, 512], F32, tag="mm_qkv")
                    for c in range(2):
                        nc.tensor.matmul(mp[:, 0, :], lhst[:, 2 * i + c, :],
                                         wqkv_sb[:, l, c, 0:512],
                                         start=(c == 0), stop=(c == 1))
                        nc.tensor.matmul(mp[:, 1, 0:D], lhst[:, 2 * i + c, :],
                                         wqkv_sb[:, l, c, 512:768],
                                         start=(c == 0), stop=(c == 1))
                    # evac to bf16: q,k contiguous; v transposed to [h, d, j]
                    nc.scalar.copy(out=qk[:, i, :], in_=mp[:, 0, :])
                    nc.scalar.copy(
                        out=vt[:, :, :, i],
                        in_=mp[:, 1, 0:D].rearrange("p (h d) -> p h d", h=H))
                if nonzero_bias:
                    for i in range(4):
                        nc.vector.tensor_add(qk[:, i, :], qk[:, i, :],
                                             bqkv_sb[:, l, 0:512])
                        nc.vector.tensor_add(
                            vt[:, :, :, i], vt[:, :, :, i],
                            bqkv_sb[:, l, 512:768].rearrange(
                                "p (h d) -> p h d", h=H))

                # ---- attention (bf16, tree reductions keep DVE in 2x mode) --
                q2 = qk[:, :, 0:D]
                k2 = qk[:, :, D:2 * D]
                with nc.allow_low_precision(reason="bf16 attention, rel-err ok"):
                    # prod[p, i, j, (h d)] = q[p,i,(h d)] * k[p,j,(h d)]
                    # (i,j,hd) order keeps every AP <=3 coalesced free dims.
                    prod = attw.tile([P, 4, 4, D], BF16, tag="att_prod")
                    qb = q2[:, :, None, :].to_broadcast((P, 4, 4, D))
                    kb = k2[:, None, :, :].to_broadcast((P, 4, 4, D))
                    nc.vector.tensor_tensor(prod, qb, kb, OP.mult)
                    # scores: halving tree over d (contiguous step-1 adds)
                    pr5 = prod.rearrange("p i j (h d) -> p i j h d", h=H)
                    s16 = attw.tile([P, 4, 4, H, 16], BF16, tag="att_s16")
                    nc.vector.tensor_tensor(s16, pr5[:, :, :, :, 0:16],
                                            pr5[:, :, :, :, 16:32], OP.add)
                    s8 = attw.tile([P, 4, 4, H, 8], BF16, tag="att_s8")
                    nc.vector.tensor_tensor(s8, s16[:, :, :, :, 0:8],
                                            s16[:, :, :, :, 8:16], OP.add)
                    s4 = attw.tile([P, 4, 4, H, 4], BF16, tag="att_s4")
                    nc.vector.tensor_tensor(s4, s8[:, :, :, :, 0:4],
                                            s8[:, :, :, :, 4:8], OP.add)
                    s2 = attw.tile([P, 4, 4, H, 2], BF16, tag="att_s2")
                    nc.vector.tensor_tensor(s2, s4[:, :, :, :, 0:2],
                                            s4[:, :, :, :, 2:4], OP.add)
                    sc = work.tile([P, 4, 4, H], BF16, tag="att_sc")
                    nc.vector.tensor_tensor(sc, s2[:, :, :, :, 0],
                                            s2[:, :, :, :, 1], OP.add)
                    # sc is [p, i, j, h]; softmax over j
                    esc = work.tile([P, 4, 4, H], BF16, tag="att_esc")
                    nc.scalar.activation(out=esc, in_=sc, func=AF.Exp, scale=SCALE)
                    den2 = work.tile([P, 4, 2, H], F32, tag="att_den2")
                    nc.vector.tensor_tensor(den2, esc[:, :, 0:2, :],
                                            esc[:, :, 2:4, :], OP.add)
                    den = work.tile([P, 4, H], F32, tag="att_den")
                    nc.vector.tensor_tensor(den, den2[:, :, 0, :],
                                            den2[:, :, 1, :], OP.add)
                    rden = work.tile([P, 4, H], F32, tag="att_rden")
                    nc.vector.reciprocal_approx_fast(out=rden, in_=den)
                    # prob_t[p, i, h, j] (transposed so pv runs step-1 in j)
                    prob = work.tile([P, 4, H, 4], BF16, tag="att_prob")
                    nc.vector.tensor_tensor(
                        prob, esc.transpose([0, 1, 3, 2]),
                        rden[:, :, :, None].to_broadcast((P, 4, H, 4)), OP.mult)
                    # pv[p, i, h, d, j] = prob_t[p,i,h,j] * vt[p,h,d,j]
                    pv = attw.tile([P, 4, H, HD, 4], BF16, tag="att_pv")
                    pb = prob[:, :, :, None, :].to_broadcast((P, 4, H, HD, 4))
                    vb = vt[:, None, :, :, :].to_broadcast((P, 4, H, HD, 4))
                    nc.vector.tensor_tensor(pv, pb, vb, OP.mult)
                    o = opool.tile([P, 4, D], BF16, tag="att_o")
                    o4 = o.rearrange("p i (h d) -> p i h d", h=H)
                    t1 = attw.tile([P, 4, H, HD, 2], BF16, tag="att_t1")
                    nc.vector.tensor_tensor(t1, pv[:, :, :, :, 0:2],
                                            pv[:, :, :, :, 2:4], OP.add)
                    nc.vector.tensor_tensor(o4, t1[:, :, :, :, 0],
                                            t1[:, :, :, :, 1], OP.add)

                # ---- o @ Wo + residual ----
                lhsto = _transpose_to_lhst(nc, pools, o.rearrange("p i d -> p (i d)"),
                                           8, "lhst")
                for i in range(4):
                    mp = mmpsum.tile([P, 512], F32, tag="mm_d")
                    for c in range(2):
                        nc.tensor.matmul(mp[:, 0:D], lhsto[:, 2 * i + c, :],
                                         wo_sb[:, l, c, :],
                                         start=(c == 0), stop=(c == 1))
                    if nonzero_bias:
                        nc.vector.tensor_add(mp[:, 0:D], mp[:, 0:D],
                                             bmisc_sb[:, l, 0, :])
                    nc.vector.tensor_add(x[:, i, :], x[:, i, :], mp[:, 0:D])

                # ---- FF block ----
                rstd, nmr = _ln_stats(nc, pools, x, 4, D)
                t2 = work.tile([P, 4, D], BF16, tag="t2_ln")
                for g in range(4):
                    nc.scalar.activation(out=t2[:, g, :], in_=x[:, g, :],
                                         func=AF.Identity,
                                         bias=nmr[:, g:g + 1], scale=rstd[:, g:g + 1])
                lhst2 = _transpose_to_lhst(nc, pools, t2.rearrange("p i d -> p (i d)"),
                                           8, "lhst")
                gl = work.tile([P, 4, FF], BF16, tag="gelu")
                for i in range(4):
                    mp = mmpsum.tile([P, 512], F32, tag="mm_d")
                    for c in range(2):
                        nc.tensor.matmul(mp[:, 0:FF], lhst2[:, 2 * i + c, :],
                                         w1_sb[:, l, c, :],
                                         start=(c == 0), stop=(c == 1))
                    if nonzero_bias:
                        nc.vector.tensor_add(mp[:, 0:FF], mp[:, 0:FF],
                                             bmisc_sb[:, l, 1, :])
                    nc.scalar.activation(out=gl[:, i, :], in_=mp[:, 0:FF], func=AF.Gelu)
                lhstg = _transpose_to_lhst(nc, pools, gl.rearrange("p i d -> p (i d)"),
                                           8, "lhst")
                for i in range(4):
                    mp = mmpsum.tile([P, 512], F32, tag="mm_d")
                    for c in range(2):
                        nc.tensor.matmul(mp[:, 0:D], lhstg[:, 2 * i + c, :],
                                         w2_sb[:, l, c, :],
                                         start=(c == 0), stop=(c == 1))
                    if nonzero_bias:
                        nc.vector.tensor_add(mp[:, 0:D], mp[:, 0:D],
                                             bmisc_sb[:, l, 2, :])
                    nc.vector.tensor_add(x[:, i, :], x[:, i, :], mp[:, 0:D])

            # ---- tail: final_ln, mean over tokens, out_ln ----
            rstd, nmr = _ln_stats(nc, pools, x, 4, D)
            xt = work.tile([P, 4, D], F32, tag="tail_xt")
            for g in range(4):
                nc.scalar.activation(out=xt[:, g, :], in_=x[:, g, :],
                                     func=AF.Identity,
                                     bias=nmr[:, g:g + 1], scale=rstd[:, g:g + 1])
            s01 = work.tile([P, 2, D], F32, tag="tail_s2")
            nc.vector.tensor_add(s01[:, 0, :], xt[:, 0, :], xt[:, 1, :])
            nc.vector.tensor_add(s01[:, 1, :], xt[:, 2, :], xt[:, 3, :])
            u = work.tile([P, D], F32, tag="tail_u")
            nc.vector.tensor_add(u, s01[:, 0, :], s01[:, 1, :])
            # u = 0.25*u*final_g + final_b
            nc.vector.scalar_tensor_tensor(
                out=u, in0=u, scalar=0.25, in1=vecs_sb[:, FING, :],
                op0=OP.mult, op1=OP.mult)
            nc.vector.tensor_add(u, u, vecs_sb[:, FINB, :])
            rstd, nmr = _ln_stats(nc, pools, u, 1, D)
            un = work.tile([P, D], F32, tag="tail_un")
            nc.scalar.activation(out=un, in_=u, func=AF.Identity,
                                 bias=nmr[:, :1], scale=rstd[:, :1])
            res = opool.tile([P, D], F32, tag="res")
            nc.vector.scalar_tensor_tensor(
                out=res, in0=un, scalar=1.0, in1=vecs_sb[:, OUTG, :],
                op0=OP.bypass, op1=OP.mult)
            nc.vector.tensor_add(res, res, vecs_sb[:, OUTB, :])
            nc.sync.dma_start(out=out[row:row + P, :], in_=res)

    return nc


def _fold_host(inputs):
    """Fold LN gains/biases into weights on the host. Returns weight arrays."""
    f = lambda k: np.asarray(inputs[k], dtype=np.float32)
    wqkv, bqkv = f("Wqkv"), f("bqkv")
    wo, bo = f("Wo"), f("bo")
    w1, b1 = f("W1"), f("b1")
    w2, b2 = f("W2"), f("b2")
    g1, b1n = f("ln1_g"), f("ln1_b")
    g2, b2n = f("ln2_g"), f("ln2_b")

    wqkv_f = np.empty_like(wqkv)
    bqkv_f = np.empty_like(bqkv)
    w1_f = np.empty_like(w1)
    b1_f = np.empty_like(b1)
    for l in range(L):
        wqkv_f[l] = g1[l][:, None] * wqkv[l]
        bqkv_f[l] = b1n[l] @ wqkv[l] + bqkv[l]
        w1_f[l] = g2[l][:, None] * w1[l]
        b1_f[l] = b2n[l] @ w1[l] + b1[l]

    symw = np.zeros((P, D), dtype=np.float32)
    symw[:SYM] = f("sym_W")
    symb = f("sym_b")

    vecs = np.zeros((9, D), dtype=np.float32)
    tte = f("token_type_emb")
    vecs[0] = f("sym_ln_g")
    vecs[1] = f("sym_ln_b") + tte[2]
    vecs[2] = tte[0]
    vecs[3] = tte[1]
    vecs[4] = tte[3]
    vecs[5] = f("final_ln_g")
    vecs[6] = f("final_ln_b")
    vecs[7] = f("out_ln_g")
    vecs[8] = f("out_ln_b")

    bmisc = np.stack([bo, b1_f, b2], axis=1)  # [L, 3, D]
    # sym_b folds into the sym matmul? sym_b is added before LN; approximate by
    # appending to symw via... keep it simple: sym matmul bias handled via vecs?
    # sym_b is a pre-LN bias: z = sf@symW + sym_b; LN removes constant shifts in
    # mean but not exactly (it does: LN(z + c) has same (z-mu) when c is constant
    # across features? No: c varies per feature). Handle nonzero sym_b via bqkv
    # path: we add it with a broadcast add if nonzero.
    nz = any(np.any(a) for a in (bqkv_f, bmisc, symb))
    return dict(symw=symw, symb=symb, wqkv=wqkv_f, bqkv=bqkv_f, wo=wo, w1=w1_f,
                w2=w2, vecs=vecs, bmisc=bmisc, nonzero_bias=bool(nz))


_CACHE = {}


def _get_built(nonzero_bias):
    key = ("k2", nonzero_bias)
    if key not in _CACHE:
        from concourse import bacc
        nc = bacc.Bacc("TRN2", target_bir_lowering=False, debug=False,
                       num_devices=NCORES)
        build_kernel(nc, nonzero_bias)
        nc.compile()
        _CACHE[key] = nc
    return _CACHE[key]


def _chunk_w(w):
    """[L, 256, M] -> [L, 2, 128, M]"""
    Lx, K, M = w.shape
    return np.ascontiguousarray(w.reshape(Lx, 2, P, M))


def kernel(**inputs):
    fold = _fold_host(inputs)
    nzb = fold["nonzero_bias"]
    if np.any(fold["symb"]):
        # rare general path: push sym_b into the padded symw via constant row:
        # append bias as an extra input feature is not available; instead add
        # sym_b to every row after matmul by augmenting... handled by adding
        # sym_b into the matmul via an extra all-ones input column (row 64).
        fold = dict(fold)
        symw = fold["symw"].copy()
        symw[SYM] = fold["symb"]
        fold["symw"] = symw
        _SYM_ONE = True
    else:
        _SYM_ONE = False

    nc = _get_built(nzb)

    ge = np.asarray(inputs["global_emb"], dtype=np.float32)
    pe = np.asarray(inputs["pert_emb"], dtype=np.float32)
    pp = np.asarray(inputs["ppi_feat"], dtype=np.float32)
    sf = np.asarray(inputs["sym_feat"], dtype=np.float32)
    if _SYM_ONE:
        # cannot append ones column without kernel change; fall back: error
        raise NotImplementedError("nonzero sym_b not supported in this build")

    wq = _chunk_w(fold["wqkv"])
    wo = _chunk_w(fold["wo"])
    w1 = _chunk_w(fold["w1"])
    w2 = _chunk_w(fold["w2"])

    in_maps = []
    for c in range(NCORES):
        sl = slice(c * BC, (c + 1) * BC)
        m = {
            "ge": np.ascontiguousarray(ge[sl]),
            "pe": np.ascontiguousarray(pe[sl]),
            "pp": np.ascontiguousarray(pp[sl]),
            "sf": np.ascontiguousarray(sf[sl]),
            "symw": fold["symw"],
            "wqkv": wq, "wo": wo, "w1": w1, "w2": w2,
            "vecs": fold["vecs"],
        }
        if nzb:
            m["bqkv"] = fold["bqkv"].astype(np.float32)
            m["bmisc"] = np.ascontiguousarray(fold["bmisc"].astype(np.float32))
        in_maps.append(m)

    res = run_bass_kernel_spmd(nc, in_maps, core_ids=list(range(NCORES)))
    global LAST_RESULT
    LAST_RESULT = res
    outs = [res.results[c]["out"] for c in range(NCORES)]
    return np.concatenate(outs, axis=0)


LAST_RESULT = None


if __name__ == "__main__":
    rng = np.random.default_rng(0)
    print("smoke build only")
    _get_built(False)
    print("built ok")



# revision 18
# speedup vs baseline: 1.6752x; 1.5210x over previous
"""Trainium2 Bass kernel for nn_CrossAttentionFusion (dense_transformer).

Strategy: pure data parallel over 8 NeuronCores (batch 32768 -> 4096/core).
Token-major layout on chip: batch rows on SBUF partitions, the 4 tokens x 256
features in the free dimension.  Dense matmuls run activation-stationary in
bf16 (fp32 PSUM accumulation, full PE rate + fast weight load); attention
(seq=4, 8 heads x 32 dims) runs on the Vector engine with broadcast access
patterns; LayerNorm uses bn_stats + ScalarE per-partition affine.  LN gains
are folded into the following weight matrices on the host; biases in this
problem are all zero but a general path applies them when nonzero.
"""

import contextlib
import ctypes
import math
import os
import sys
import types
from contextlib import ExitStack

import numpy as np

import concourse.bass as bass
import concourse.tile as tile
from concourse import mybir
from concourse.bass_utils import run_bass_kernel_spmd
from concourse.masks import make_identity


def _install_ntff_hook_shim():
    """Provide antenv.axon_hooks if the image lacks it, so trace=True works.

    Mirrors trn_agent_boot._ntff_profile_via_ctypes: drives NTFF capture via
    the axon PJRT .so's C ABI.  No-op if the real module exists.
    """
    try:
        import antenv.axon_hooks  # noqa: F401
        return
    except ImportError:
        pass
    so_path = "/opt/axon/libaxon_pjrt.so"
    hook = None
    if os.path.exists(so_path):
        try:
            lib = ctypes.CDLL(so_path)
            if hasattr(lib, "axon_start_nrt_profile"):
                lib.axon_start_nrt_profile.argtypes = [
                    ctypes.POINTER(ctypes.c_int64), ctypes.c_size_t]
                lib.axon_start_nrt_profile.restype = ctypes.c_int64
                lib.axon_stop_nrt_profile.argtypes = [ctypes.c_char_p]
                lib.axon_stop_nrt_profile.restype = ctypes.c_int64

                @contextlib.contextmanager
                def _hook(output_dir, device_ids):
                    import jax
                    jax.devices()
                    if device_ids:
                        ids = (ctypes.c_int64 * len(device_ids))(*device_ids)
                        rc = lib.axon_start_nrt_profile(ids, len(device_ids))
                    else:
                        rc = lib.axon_start_nrt_profile(None, 0)
                    if rc != 0:
                        raise RuntimeError(f"axon_start_nrt_profile rc={rc}")
                    try:
                        yield
                    finally:
                        n = lib.axon_stop_nrt_profile(str(output_dir).encode())
                        print(f"ntff profile: {n} file(s) -> {output_dir}",
                              file=sys.stderr)

                hook = _hook
        except OSError:
            pass

    mod = types.ModuleType("antenv.axon_hooks")
    mod.get_axon_ntff_profile_hook = lambda: hook
    mod.set_axon_ntff_profile_hook = lambda h: None
    sys.modules["antenv.axon_hooks"] = mod


_install_ntff_hook_shim()

# Problem shapes (hardcoded per contract).
D, H, HD, FF, L, SYM, B = 256, 8, 32, 256, 3, 64, 32768
NCORES = 8
BC = B // NCORES          # 4096 rows per core
P = 128                   # SBUF partitions
NT = BC // P              # 32 tiles per core
F32 = mybir.dt.float32
F32R = mybir.dt.float32r
BF16 = mybir.dt.bfloat16
AF = mybir.ActivationFunctionType
OP = mybir.AluOpType
EPS = 1e-5
SCALE = 1.0 / math.sqrt(HD)


def _r(ap):
    """View an fp32 AP as float32r for full-rate PE matmuls."""
    return ap.bitcast(F32R)


def _ln_stats(nc, pools, x_ap, ngroups, gsize, psrc=False):
    """Return (rstd [P,ngroups], neg_mu_rstd [P,ngroups]) for LN over gsize.

    x_ap: [P, ngroups, gsize] (or [P, gsize] if ngroups==1).
    """
    work = pools["work"]
    stats = work.tile([P, ngroups, 6], F32, tag="ln_stats")
    if ngroups == 1:
        nc.vector.bn_stats(out=stats[:, 0, :], in_=x_ap)
    else:
        # walrus requires bn_stats output to be exactly 6 elems/partition
        for g in range(ngroups):
            nc.vector.bn_stats(out=stats[:, g, :], in_=x_ap[:, g, :])
    mv = work.tile([P, ngroups, 2], F32, tag="ln_mv")
    for g in range(ngroups):
        nc.vector.bn_aggr(out=mv[:, g, :], in_=stats[:, g, :])
    # rstd = 1/sqrt(var + eps)
    rstd = work.tile([P, ngroups], F32, tag="ln_rstd")
    nc.scalar.activation(
        out=rstd, in_=mv[:, :, 1], func=AF.Sqrt, bias=pools["eps"][:, :1], scale=1.0
    )
    nc.vector.reciprocal(out=rstd, in_=rstd)
    # neg_mu_rstd = -(mu * rstd)
    nmr = work.tile([P, ngroups], F32, tag="ln_nmr")
    nc.vector.scalar_tensor_tensor(
        out=nmr, in0=mv[:, :, 0], scalar=-1.0, in1=rstd, op0=OP.mult, op1=OP.mult
    )
    return rstd, nmr


def _transpose_to_lhst(nc, pools, src_ap, nchunks, tag):
    """PE-transpose src_ap [P, nchunks*128] (fp32) -> SBUF lhsT [128, nchunks, 128].

    Returns the SBUF tile holding x^T chunks: lhsT[:, c, :] = src[:, c*128:(c+1)*128].T
    """
    tp = pools["tpsum"]
    lhst = pools["lhst"].tile([P, nchunks, P], BF16, tag=tag)
    for c0 in range(0, nchunks, 4):
        cn = min(4, nchunks - c0)
        pt = tp.tile([P, 4, P], BF16, tag="tpsum")
        for c in range(cn):
            nc.tensor.transpose(
                pt[:, c, :], src_ap[:, (c0 + c) * P:(c0 + c + 1) * P], pools["identb"]
            )
        # bf16 PSUM->SBUF on DVE (2x_1P) keeps the scalar engine free
        nc.vector.tensor_copy(out=lhst[:, c0:c0 + cn, :], in_=pt[:, :cn, :])
    return lhst


def build_kernel(nc, nonzero_bias):
    """Trace the full forward pass for one core (BC rows)."""
    # Per-core data inputs.
    ge = nc.dram_tensor("ge", [BC, D], F32, kind="ExternalInput").ap()
    pe = nc.dram_tensor("pe", [BC, D], F32, kind="ExternalInput").ap()
    pp = nc.dram_tensor("pp", [BC, D], F32, kind="ExternalInput").ap()
    sf = nc.dram_tensor("sf", [BC, SYM], F32, kind="ExternalInput").ap()
    # Folded weights (replicated).
    symw = nc.dram_tensor("symw", [P, D], F32, kind="ExternalInput").ap()  # padded 64->128
    wqkv = nc.dram_tensor("wqkv", [L, 2, P, 3 * D], F32, kind="ExternalInput").ap()
    wo = nc.dram_tensor("wo", [L, 2, P, D], F32, kind="ExternalInput").ap()
    w1 = nc.dram_tensor("w1", [L, 2, P, FF], F32, kind="ExternalInput").ap()
    w2 = nc.dram_tensor("w2", [L, 2, P, D], F32, kind="ExternalInput").ap()
    # Vectors: packed [n, D] table: sym_g, sym_b(+tte2), tte0, tte1, tte3,
    # final_g, final_b, out_g, out_b
    vecs = nc.dram_tensor("vecs", [9, D], F32, kind="ExternalInput").ap()
    bqkv = bmisc = None
    if nonzero_bias:
        bqkv = nc.dram_tensor("bqkv", [L, 3 * D], F32, kind="ExternalInput").ap()
        bmisc = nc.dram_tensor("bmisc", [L, 3, D], F32, kind="ExternalInput").ap()
    out = nc.dram_tensor("out", [BC, D], F32, kind="ExternalOutput").ap()

    with ExitStack() as ctx:
        tc = ctx.enter_context(tile.TileContext(nc))
        singles = ctx.enter_context(tc.tile_pool(name="singles", bufs=1))
        work = ctx.enter_context(tc.tile_pool(name="work", bufs=3))
        xpool = ctx.enter_context(tc.tile_pool(name="xpool", bufs=3))
        qkvpool = ctx.enter_context(tc.tile_pool(name="qkvpool", bufs=2))
        lhstp = ctx.enter_context(tc.tile_pool(name="lhst", bufs=3))
        tpsum = ctx.enter_context(tc.tile_pool(name="tpsum", bufs=2, space="PSUM"))
        mmpsum = ctx.enter_context(tc.tile_pool(name="mmpsum", bufs=2, space="PSUM"))
        opool = ctx.enter_context(tc.tile_pool(name="opool", bufs=2))
        attw = ctx.enter_context(tc.tile_pool(name="attw", bufs=2))

        # ---- load constants ----
        ident = singles.tile([P, P], F32)
        make_identity(nc, ident)
        identb = singles.tile([P, P], BF16)
        make_identity(nc, identb)
        eps_t = singles.tile([P, 1], F32)
        nc.vector.memset(eps_t, EPS)
        symw_sb = singles.tile([P, D], BF16)
        nc.gpsimd.dma_start(out=symw_sb, in_=symw)
        wqkv_sb = singles.tile([P, L, 2, 3 * D], BF16)
        nc.gpsimd.dma_start(out=wqkv_sb, in_=wqkv.transpose([2, 0, 1, 3]))
        wo_sb = singles.tile([P, L, 2, D], BF16)
        nc.gpsimd.dma_start(out=wo_sb, in_=wo.transpose([2, 0, 1, 3]))
        w1_sb = singles.tile([P, L, 2, FF], BF16)
        nc.gpsimd.dma_start(out=w1_sb, in_=w1.transpose([2, 0, 1, 3]))
        w2_sb = singles.tile([P, L, 2, D], BF16)
        nc.gpsimd.dma_start(out=w2_sb, in_=w2.transpose([2, 0, 1, 3]))
        vecs_sb = singles.tile([P, 9, D], F32)
        nc.sync.dma_start(out=vecs_sb, in_=vecs.partition_broadcast(P))
        bqkv_sb = bmisc_sb = None
        if nonzero_bias:
            bqkv_sb = singles.tile([P, L, 3 * D], F32)
            nc.sync.dma_start(out=bqkv_sb, in_=bqkv.partition_broadcast(P))
            bmisc_sb = singles.tile([P, L, 3, D], F32)
            nc.sync.dma_start(out=bmisc_sb, in_=bmisc.partition_broadcast(P))

        pools = {
            "work": work, "tpsum": tpsum, "lhst": lhstp,
            "ident": ident, "identb": identb, "eps": eps_t,
        }
        SYMG, SYMBT, TTE0, TTE1, TTE3 = 0, 1, 2, 3, 4
        FING, FINB, OUTG, OUTB = 5, 6, 7, 8

        def tile_body(it):
            """Generator: yields at stage boundaries so two tiles can be
            interleaved instruction-stream-wise (keeps every engine queue
            loaded with independent work -> cross-tile overlap, PE stays
            warm)."""
            row = it * P
            # ---- build x [P, 4, D] ----
            x = xpool.tile([P, 4, D], F32, tag="x")
            ine = work.tile([P, 3, D], F32, tag="ine")
            nc.sync.dma_start(out=ine[:, 0, :], in_=ge[row:row + P, :])
            nc.sync.dma_start(out=ine[:, 1, :], in_=pe[row:row + P, :])
            nc.sync.dma_start(out=ine[:, 2, :], in_=pp[row:row + P, :])
            sft = work.tile([P, SYM], F32, tag="sft")
            nc.sync.dma_start(out=sft, in_=sf[row:row + P, :])
            yield

            nc.vector.tensor_add(x[:, 0, :], ine[:, 0, :], vecs_sb[:, TTE0, :])
            nc.vector.tensor_add(x[:, 1, :], ine[:, 1, :], vecs_sb[:, TTE1, :])
            nc.vector.tensor_add(x[:, 3, :], ine[:, 2, :], vecs_sb[:, TTE3, :])

            # sym branch: LN(sf @ symW) * g + (b + tte2)
            sftp = work.tile([P, P], BF16, tag="sftp")
            nc.vector.memset(sftp[:, SYM:], 0.0)
            nc.vector.tensor_copy(out=sftp[:, :SYM], in_=sft)
            spsum_t = tpsum.tile([P, 4, P], BF16, tag="tpsum", name="spsum")
            spsum = spsum_t[:, 0, :]
            nc.tensor.transpose(spsum, sftp, identb)
            slhst = work.tile([P, P], BF16, tag="slhst")
            nc.scalar.copy(out=slhst, in_=spsum)
            zsym_t = mmpsum.tile([P, 512], F32, tag="mm_d", name="zsym")
            zsym = zsym_t[:, 0:D]
            nc.tensor.matmul(zsym, slhst, symw_sb, start=True, stop=True)
            rstd, nmr = _ln_stats(nc, pools, zsym, 1, D)
            zn = work.tile([P, D], F32, tag="zn")
            nc.scalar.activation(out=zn, in_=zsym, func=AF.Identity,
                                 bias=nmr[:, :1], scale=rstd[:, :1])
            # x2 = zn * symg + (symb + tte2)
            nc.vector.scalar_tensor_tensor(
                out=x[:, 2, :], in0=zn, scalar=1.0, in1=vecs_sb[:, SYMG, :],
                op0=OP.bypass, op1=OP.mult)
            nc.vector.tensor_add(x[:, 2, :], x[:, 2, :], vecs_sb[:, SYMBT, :])
            yield

            # ---- transformer layers ----
            for l in range(L):
                # LN1 (gains folded into wqkv)
                rstd, nmr = _ln_stats(nc, pools, x, 4, D)
                t = work.tile([P, 4, D], BF16, tag="t_ln")
                for g in range(4):
                    nc.scalar.activation(out=t[:, g, :], in_=x[:, g, :],
                                         func=AF.Identity,
                                         bias=nmr[:, g:g + 1], scale=rstd[:, g:g + 1])
                yield
                # qkv = t @ wqkv  (activation-stationary).  Each lhsT chunk is
                # loaded once and reused for both output slices (LDW reuse).
                qk = qkvpool.tile([P, 4, 512], BF16, tag="qk")
                vt = qkvpool.tile([P, H, HD, 4], BF16, tag="vt")
                lhst = _transpose_to_lhst(nc, pools, t.rearrange("p i d -> p (i d)"),
                                          8, "lhst")
                yield
                for i in range(4):
                    mp = mmpsum.tile([P, 2, 512], F32, tag="mm_qkv")
                    for c in range(2):
                        nc.tensor.matmul(mp[:, 0, :], lhst[:, 2 * i + c, :],
                                         wqkv_sb[:, l, c, 0:512],
                                         start=(c == 0), stop=(c == 1))
                        nc.tensor.matmul(mp[:, 1, 0:D], lhst[:, 2 * i + c, :],
                                         wqkv_sb[:, l, c, 512:768],
                                         start=(c == 0), stop=(c == 1))
                    # evac to bf16: q,k contiguous; v transposed to [h, d, j]
                    nc.scalar.copy(out=qk[:, i, :], in_=mp[:, 0, :])
                    nc.scalar.copy(
                        out=vt[:, :, :, i],
                        in_=mp[:, 1, 0:D].rearrange("p (h d) -> p h d", h=H))
                if nonzero_bias:
                    for i in range(4):
                        nc.vector.tensor_add(qk[:, i, :], qk[:, i, :],
                                             bqkv_sb[:, l, 0:512])
                        nc.vector.tensor_add(
                            vt[:, :, :, i], vt[:, :, :, i],
                            bqkv_sb[:, l, 512:768].rearrange(
                                "p (h d) -> p h d", h=H))
                yield

                # ---- attention (bf16, tree reductions keep DVE in 2x mode) --
                q2 = qk[:, :, 0:D]
                k2 = qk[:, :, D:2 * D]
                with nc.allow_low_precision(reason="bf16 attention, rel-err ok"):
                    # prod[p, i, j, (h d)] = q[p,i,(h d)] * k[p,j,(h d)]
                    # (i,j,hd) order keeps every AP <=3 coalesced free dims.
                    prod = attw.tile([P, 4, 4, D], BF16, tag="att_prod")
                    qb = q2[:, :, None, :].to_broadcast((P, 4, 4, D))
                    kb = k2[:, None, :, :].to_broadcast((P, 4, 4, D))
                    nc.vector.tensor_tensor(prod, qb, kb, OP.mult)
                    # scores: halving tree over d (contiguous step-1 adds)
                    pr5 = prod.rearrange("p i j (h d) -> p i j h d", h=H)
                    s16 = attw.tile([P, 4, 4, H, 16], BF16, tag="att_s16")
                    nc.vector.tensor_tensor(s16, pr5[:, :, :, :, 0:16],
                                            pr5[:, :, :, :, 16:32], OP.add)
                    s8 = attw.tile([P, 4, 4, H, 8], BF16, tag="att_s8")
                    nc.vector.tensor_tensor(s8, s16[:, :, :, :, 0:8],
                                            s16[:, :, :, :, 8:16], OP.add)
                    s4 = attw.tile([P, 4, 4, H, 4], BF16, tag="att_s4")
                    nc.vector.tensor_tensor(s4, s8[:, :, :, :, 0:4],
                                            s8[:, :, :, :, 4:8], OP.add)
                    s2 = attw.tile([P, 4, 4, H, 2], BF16, tag="att_s2")
                    nc.vector.tensor_tensor(s2, s4[:, :, :, :, 0:2],
                                            s4[:, :, :, :, 2:4], OP.add)
                    sc = work.tile([P, 4, 4, H], BF16, tag="att_sc")
                    nc.vector.tensor_tensor(sc, s2[:, :, :, :, 0],
                                            s2[:, :, :, :, 1], OP.add)
                    # sc is [p, i, j, h]; softmax over j
                    esc = work.tile([P, 4, 4, H], BF16, tag="att_esc")
                    nc.scalar.activation(out=esc, in_=sc, func=AF.Exp, scale=SCALE)
                    den2 = work.tile([P, 4, 2, H], F32, tag="att_den2")
                    nc.vector.tensor_tensor(den2, esc[:, :, 0:2, :],
                                            esc[:, :, 2:4, :], OP.add)
                    den = work.tile([P, 4, H], F32, tag="att_den")
                    nc.vector.tensor_tensor(den, den2[:, :, 0, :],
                                            den2[:, :, 1, :], OP.add)
                    rden = work.tile([P, 4, H], F32, tag="att_rden")
                    nc.vector.reciprocal_approx_fast(out=rden, in_=den)
                    # prob_t[p, i, h, j] (transposed so pv runs step-1 in j)
                    prob = work.tile([P, 4, H, 4], BF16, tag="att_prob")
                    nc.vector.tensor_tensor(
                        prob, esc.transpose([0, 1, 3, 2]),
                        rden[:, :, :, None].to_broadcast((P, 4, H, 4)), OP.mult)
                    # pv[p, i, h, d, j] = prob_t[p,i,h,j] * vt[p,h,d,j]
                    pv = attw.tile([P, 4, H, HD, 4], BF16, tag="att_pv")
                    pb = prob[:, :, :, None, :].to_broadcast((P, 4, H, HD, 4))
                    vb = vt[:, None, :, :, :].to_broadcast((P, 4, H, HD, 4))
                    nc.vector.tensor_tensor(pv, pb, vb, OP.mult)
                    o = opool.tile([P, 4, D], BF16, tag="att_o")
                    o4 = o.rearrange("p i (h d) -> p i h d", h=H)
                    t1 = attw.tile([P, 4, H, HD, 2], BF16, tag="att_t1")
                    nc.vector.tensor_tensor(t1, pv[:, :, :, :, 0:2],
                                            pv[:, :, :, :, 2:4], OP.add)
                    nc.vector.tensor_tensor(o4, t1[:, :, :, :, 0],
                                            t1[:, :, :, :, 1], OP.add)
                yield

                # ---- o @ Wo + residual ----
                lhsto = _transpose_to_lhst(nc, pools, o.rearrange("p i d -> p (i d)"),
                                           8, "lhst")
                yield
                for i in range(4):
                    mp = mmpsum.tile([P, 512], F32, tag="mm_d")
                    for c in range(2):
                        nc.tensor.matmul(mp[:, 0:D], lhsto[:, 2 * i + c, :],
                                         wo_sb[:, l, c, :],
                                         start=(c == 0), stop=(c == 1))
                    if nonzero_bias:
                        nc.vector.tensor_add(mp[:, 0:D], mp[:, 0:D],
                                             bmisc_sb[:, l, 0, :])
                    nc.vector.tensor_add(x[:, i, :], x[:, i, :], mp[:, 0:D])
                yield

                # ---- FF block ----
                rstd, nmr = _ln_stats(nc, pools, x, 4, D)
                t2 = work.tile([P, 4, D], BF16, tag="t2_ln")
                for g in range(4):
                    nc.scalar.activation(out=t2[:, g, :], in_=x[:, g, :],
                                         func=AF.Identity,
                                         bias=nmr[:, g:g + 1], scale=rstd[:, g:g + 1])
                yield
                lhst2 = _transpose_to_lhst(nc, pools, t2.rearrange("p i d -> p (i d)"),
                                           8, "lhst")
                gl = work.tile([P, 4, FF], BF16, tag="gelu")
                for i in range(4):
                    mp = mmpsum.tile([P, 512], F32, tag="mm_d")
                    for c in range(2):
                        nc.tensor.matmul(mp[:, 0:FF], lhst2[:, 2 * i + c, :],
                                         w1_sb[:, l, c, :],
                                         start=(c == 0), stop=(c == 1))
                    if nonzero_bias:
                        nc.vector.tensor_add(mp[:, 0:FF], mp[:, 0:FF],
                                             bmisc_sb[:, l, 1, :])
                    nc.scalar.activation(out=gl[:, i, :], in_=mp[:, 0:FF], func=AF.Gelu)
                yield
                lhstg = _transpose_to_lhst(nc, pools, gl.rearrange("p i d -> p (i d)"),
                                           8, "lhst")
                for i in range(4):
                    mp = mmpsum.tile([P, 512], F32, tag="mm_d")
                    for c in range(2):
                        nc.tensor.matmul(mp[:, 0:D], lhstg[:, 2 * i + c, :],
                                         w2_sb[:, l, c, :],
                                         start=(c == 0), stop=(c == 1))
                    if nonzero_bias:
                        nc.vector.tensor_add(mp[:, 0:D], mp[:, 0:D],
                                             bmisc_sb[:, l, 2, :])
                    nc.vector.tensor_add(x[:, i, :], x[:, i, :], mp[:, 0:D])
                yield

            # ---- tail: final_ln, mean over tokens, out_ln ----
            rstd, nmr = _ln_stats(nc, pools, x, 4, D)
            xt = work.tile([P, 4, D], F32, tag="tail_xt")
            for g in range(4):
                nc.scalar.activation(out=xt[:, g, :], in_=x[:, g, :],
                                     func=AF.Identity,
                                     bias=nmr[:, g:g + 1], scale=rstd[:, g:g + 1])
            s01 = work.tile([P, 2, D], F32, tag="tail_s2")
            nc.vector.tensor_add(s01[:, 0, :], xt[:, 0, :], xt[:, 1, :])
            nc.vector.tensor_add(s01[:, 1, :], xt[:, 2, :], xt[:, 3, :])
            u = work.tile([P, D], F32, tag="tail_u")
            nc.vector.tensor_add(u, s01[:, 0, :], s01[:, 1, :])
            # u = 0.25*u*final_g + final_b
            nc.vector.scalar_tensor_tensor(
                out=u, in0=u, scalar=0.25, in1=vecs_sb[:, FING, :],
                op0=OP.mult, op1=OP.mult)
            nc.vector.tensor_add(u, u, vecs_sb[:, FINB, :])
            rstd, nmr = _ln_stats(nc, pools, u, 1, D)
            un = work.tile([P, D], F32, tag="tail_un")
            nc.scalar.activation(out=un, in_=u, func=AF.Identity,
                                 bias=nmr[:, :1], scale=rstd[:, :1])
            res = opool.tile([P, D], F32, tag="res")
            nc.vector.scalar_tensor_tensor(
                out=res, in0=un, scalar=1.0, in1=vecs_sb[:, OUTG, :],
                op0=OP.bypass, op1=OP.mult)
            nc.vector.tensor_add(res, res, vecs_sb[:, OUTB, :])
            nc.sync.dma_start(out=out[row:row + P, :], in_=res)
            yield

        # Drive pairs of tiles round-robin: alternate emission between the
        # two generators so each engine's in-order queue interleaves
        # independent work (cross-tile overlap; PE avoids HAM cool-down).
        for it0 in range(0, NT, 2):
            gens = [tile_body(it0)]
            if it0 + 1 < NT:
                gens.append(tile_body(it0 + 1))
            while gens:
                for g in list(gens):
                    try:
                        next(g)
                    except StopIteration:
                        gens.remove(g)

    return nc


def _fold_host(inputs):
    """Fold LN gains/biases into weights on the host. Returns weight arrays."""
    f = lambda k: np.asarray(inputs[k], dtype=np.float32)
    wqkv, bqkv = f("Wqkv"), f("bqkv")
    wo, bo = f("Wo"), f("bo")
    w1, b1 = f("W1"), f("b1")
    w2, b2 = f("W2"), f("b2")
    g1, b1n = f("ln1_g"), f("ln1_b")
    g2, b2n = f("ln2_g"), f("ln2_b")

    wqkv_f = np.empty_like(wqkv)
    bqkv_f = np.empty_like(bqkv)
    w1_f = np.empty_like(w1)
    b1_f = np.empty_like(b1)
    for l in range(L):
        wqkv_f[l] = g1[l][:, None] * wqkv[l]
        bqkv_f[l] = b1n[l] @ wqkv[l] + bqkv[l]
        w1_f[l] = g2[l][:, None] * w1[l]
        b1_f[l] = b2n[l] @ w1[l] + b1[l]

    symw = np.zeros((P, D), dtype=np.float32)
    symw[:SYM] = f("sym_W")
    symb = f("sym_b")

    vecs = np.zeros((9, D), dtype=np.float32)
    tte = f("token_type_emb")
    vecs[0] = f("sym_ln_g")
    vecs[1] = f("sym_ln_b") + tte[2]
    vecs[2] = tte[0]
    vecs[3] = tte[1]
    vecs[4] = tte[3]
    vecs[5] = f("final_ln_g")
    vecs[6] = f("final_ln_b")
    vecs[7] = f("out_ln_g")
    vecs[8] = f("out_ln_b")

    bmisc = np.stack([bo, b1_f, b2], axis=1)  # [L, 3, D]
    # sym_b folds into the sym matmul? sym_b is added before LN; approximate by
    # appending to symw via... keep it simple: sym matmul bias handled via vecs?
    # sym_b is a pre-LN bias: z = sf@symW + sym_b; LN removes constant shifts in
    # mean but not exactly (it does: LN(z + c) has same (z-mu) when c is constant
    # across features? No: c varies per feature). Handle nonzero sym_b via bqkv
    # path: we add it with a broadcast add if nonzero.
    nz = any(np.any(a) for a in (bqkv_f, bmisc, symb))
    return dict(symw=symw, symb=symb, wqkv=wqkv_f, bqkv=bqkv_f, wo=wo, w1=w1_f,
                w2=w2, vecs=vecs, bmisc=bmisc, nonzero_bias=bool(nz))


_CACHE = {}


def _get_built(nonzero_bias):
    key = ("k2", nonzero_bias)
    if key not in _CACHE:
        from concourse import bacc
        nc = bacc.Bacc("TRN2", target_bir_lowering=False, debug=False,
                       num_devices=NCORES)
        build_kernel(nc, nonzero_bias)
        nc.compile()
        _CACHE[key] = nc
    return _CACHE[key]


def _chunk_w(w):
    """[L, 256, M] -> [L, 2, 128, M]"""
    Lx, K, M = w.shape
    return np.ascontiguousarray(w.reshape(Lx, 2, P, M))


def kernel(**inputs):
    fold = _fold_host(inputs)
    nzb = fold["nonzero_bias"]
    if np.any(fold["symb"]):
        # rare general path: push sym_b into the padded symw via constant row:
        # append bias as an extra input feature is not available; instead add
        # sym_b to every row after matmul by augmenting... handled by adding
        # sym_b into the matmul via an extra all-ones input column (row 64).
        fold = dict(fold)
        symw = fold["symw"].copy()
        symw[SYM] = fold["symb"]
        fold["symw"] = symw
        _SYM_ONE = True
    else:
        _SYM_ONE = False

    nc = _get_built(nzb)

    ge = np.asarray(inputs["global_emb"], dtype=np.float32)
    pe = np.asarray(inputs["pert_emb"], dtype=np.float32)
    pp = np.asarray(inputs["ppi_feat"], dtype=np.float32)
    sf = np.asarray(inputs["sym_feat"], dtype=np.float32)
    if _SYM_ONE:
        # cannot append ones column without kernel change; fall back: error
        raise NotImplementedError("nonzero sym_b not supported in this build")

    wq = _chunk_w(fold["wqkv"])
    wo = _chunk_w(fold["wo"])
    w1 = _chunk_w(fold["w1"])
    w2 = _chunk_w(fold["w2"])

    in_maps = []
    for c in range(NCORES):
        sl = slice(c * BC, (c + 1) * BC)
        m = {
            "ge": np.ascontiguousarray(ge[sl]),
            "pe": np.ascontiguousarray(pe[sl]),
            "pp": np.ascontiguousarray(pp[sl]),
            "sf": np.ascontiguousarray(sf[sl]),
            "symw": fold["symw"],
            "wqkv": wq, "wo": wo, "w1": w1, "w2": w2,
            "vecs": fold["vecs"],
        }
        if nzb:
            m["bqkv"] = fold["bqkv"].astype(np.float32)
            m["bmisc"] = np.ascontiguousarray(fold["bmisc"].astype(np.float32))
        in_maps.append(m)

    res = run_bass_kernel_spmd(nc, in_maps, core_ids=list(range(NCORES)))
    global LAST_RESULT
    LAST_RESULT = res
    outs = [res.results[c]["out"] for c in range(NCORES)]
    return np.concatenate(outs, axis=0)


LAST_RESULT = None


if __name__ == "__main__":
    rng = np.random.default_rng(0)
    print("smoke build only")
    _get_built(False)
    print("built ok")

